# revision 29
# baseline (speedup 1.0000x reference)
"""Dim6RotStructureHead Trainium2 kernel.

Sharding: pure data parallel — 4096 tokens split 512/core across 8 cores,
all params replicated.

Per-core pipeline (hT layout: features on partitions, tokens on free axis):
  mm1:  h_pre[e,t] = sum_d w1T[d,e] . xT[d,t]      (fp32r PE, 12x12 blocks)
  gelu: g = Gelu(h_pre + b1)                        (ACT, fused bias)
  gsq = g*g                                         (DVE)
  mm2:  q[o,t]  = sum_e (w2*ln_g)T[e,o] . g[e,t]   (+ ones col -> S1)
  s2:   S2[t]   = sum_e gsq[e,t]                    (ones lhsT)
  LayerNorm deferred past mm2: p = (q - mu*c1) * inv_sigma (+ c3); the
  direction vectors are normalized scale-invariantly:
  xv = pp/(|pp| + eps*sigma) with pp = q - mu*c1.
  Small PE transposes bring [24,512]+[1,512] -> [128 tok, 25]; geometry
  epilogue (Gram-Schmidt simplifies: trans cancels -> GS(-xv, yv), mask,
  compose, backbone coords) runs batched on DVE + ACT sqrt.
"""
import numpy as np

import concourse.bacc as bacc
import concourse.mybir as mybir
import concourse.tile as tile
from concourse.bass_utils import run_bass_kernel_spmd

F32 = mybir.dt.float32
F32R = mybir.dt.float32r
U32 = mybir.dt.uint32
AF = mybir.ActivationFunctionType
ALU = mybir.AluOpType
AX = mybir.AxisListType
QMAGIC = 0x5F3759DF

B, L, D = 4, 1024, 1536
NCORES = 8
TOK = (B * L) // NCORES          # 512 tokens per core
KD = D // 128                    # 12 contraction blocks
KE = D // 128                    # 12 feature blocks
NB = TOK // 128                  # 4 token blocks per core
OUT2 = 24                        # 23 proj outputs + ones column (S1)
TRANS_SCALE = 10.0
LN_EPS = 1e-5
NORM_EPS = 1e-5
GS_EPS = 1e-12


def _bc(ap, axis, n):
    """Broadcast AP along a new axis (step-0)."""
    ap = ap.unsqueeze(axis)
    shape = list(ap.shape)
    shape[axis] = n
    return ap.broadcast_to(shape)


def build_program(c3_nonzero: bool):
    nc = bacc.Bacc("TRN2", target_bir_lowering=False, debug=False,
                   enable_asserts=False, num_devices=NCORES)

    # ---- DRAM I/O (per core) ----
    # x: [p][kd][t]; loaded per-kd (2KB contiguous per partition per DMA)
    xr_d = nc.dram_tensor("xr", [128, KD, TOK], F32R, kind="ExternalInput").ap()
    w1r_d = nc.dram_tensor("w1r", [KE, KD, 128, 128], F32R, kind="ExternalInput").ap()
    w2c_d = nc.dram_tensor("w2c", [128, KE, OUT2], F32R, kind="ExternalInput").ap()
    ones_d = nc.dram_tensor("ones1", [128, 1], F32R, kind="ExternalInput").ap()
    b1c_d = nc.dram_tensor("b1c", [128, KE], F32, kind="ExternalInput").ap()
    c1b_d = nc.dram_tensor("c1b", [128, 23], F32, kind="ExternalInput").ap()
    c3b_d = nc.dram_tensor("c3b", [128, 23], F32, kind="ExternalInput").ap()
    i9b_d = nc.dram_tensor("i9b", [128, 9], F32, kind="ExternalInput").ap()
    ident_d = nc.dram_tensor("ident", [128, 128], F32, kind="ExternalInput").ap()
    aff_d = nc.dram_tensor("aff", [128, NB, 12], F32, kind="ExternalInput").ap()
    msk_d = nc.dram_tensor("maskf", [128, NB], F32, kind="ExternalInput").ap()
    affo_d = nc.dram_tensor("aff_out", [128, NB, 12], F32, kind="ExternalOutput").ap()
    xyzo_d = nc.dram_tensor("xyz_out", [128, NB, 9], F32, kind="ExternalOutput").ap()

    with tile.TileContext(nc) as tc:
        with tc.tile_pool(name="wgt", bufs=1) as wgt, \
             tc.tile_pool(name="xin", bufs=1) as xin, \
             tc.tile_pool(name="cst", bufs=1) as cst, \
             tc.tile_pool(name="gp", bufs=3) as gp, \
             tc.tile_pool(name="sq", bufs=3) as sqp, \
             tc.tile_pool(name="ep", bufs=1) as ep, \
             tc.tile_pool(name="ph", bufs=3, space="PSUM") as php, \
             tc.tile_pool(name="pq", bufs=1, space="PSUM") as pqp, \
             tc.tile_pool(name="ps2", bufs=1, space="PSUM") as ps2p, \
             tc.tile_pool(name="ptr", bufs=2, space="PSUM") as ptrp:

            # ---- small/replicated inputs ----
            w2c_sb = cst.tile([128, KE, OUT2], F32R, tag="w2c")
            nc.sync.dma_start(w2c_sb[:], w2c_d[:])
            ones_sb = cst.tile([128, 1], F32R, tag="ones")
            nc.sync.dma_start(ones_sb[:], ones_d[:])
            b1_sb = cst.tile([128, KE], F32, tag="b1")
            nc.sync.dma_start(b1_sb[:], b1c_d[:])
            c1b_sb = cst.tile([128, 23], F32, tag="c1b")
            nc.sync.dma_start(c1b_sb[:], c1b_d[:])
            c3b_sb = cst.tile([128, 23], F32, tag="c3b")
            if c3_nonzero:
                nc.sync.dma_start(c3b_sb[:], c3b_d[:])
            i9b_sb = cst.tile([128, 9], F32, tag="i9b")
            nc.sync.dma_start(i9b_sb[:], i9b_d[:])
            id_sb = cst.tile([128, 128], F32, tag="ident")
            nc.sync.dma_start(id_sb[:], ident_d[:])
            aff_sb = cst.tile([128, NB, 12], F32, tag="aff")
            nc.sync.dma_start(aff_sb[:], aff_d[:])
            msk_sb = cst.tile([128, NB], F32, tag="msk")
            nc.sync.dma_start(msk_sb[:], msk_d[:])

            # ---- big inputs: w1 block 0, then x per-kd, then w1 blocks 1.. ----
            x_sb = xin.tile([128, KD, TOK], F32R, tag="x")
            w_sb = wgt.tile([128, KE, KD, 128], F32R, tag="w1")
            nc.sync.dma_start(w_sb[:, 0], w1r_d[0].transpose([1, 0, 2]))
            for kd in range(KD):
                nc.sync.dma_start(x_sb[:, kd, :], xr_d[:, kd, :])
            for ke in range(1, KE):
                nc.sync.dma_start(w_sb[:, ke], w1r_d[ke].transpose([1, 0, 2]))

            # ---- main loop ----
            pq = pqp.tile([OUT2, TOK], F32, tag="pq")
            ps2 = ps2p.tile([1, TOK], F32, tag="ps2")
            g_tiles = {}
            gsq_tiles = {}

            def mm2_pair(ke):
                nc.tensor.matmul(pq[:], w2c_sb[:, ke, :], g_tiles.pop(ke)[:],
                                 start=(ke == 0), stop=(ke == KE - 1))
                nc.tensor.matmul(ps2[:], ones_sb[:], gsq_tiles.pop(ke)[:],
                                 start=(ke == 0), stop=(ke == KE - 1))

            for ke in range(KE):
                ph = php.tile([128, TOK], F32, tag="ph")
                for kd in range(KD):
                    nc.tensor.matmul(ph[:], w_sb[:, ke, kd, :], x_sb[:, kd, :],
                                     start=(kd == 0), stop=(kd == KD - 1))
                g = gp.tile([128, TOK], F32R, tag="g")
                nc.scalar.activation(g[:], ph[:], AF.Gelu, bias=b1_sb[:, ke:ke + 1])
                gsq = sqp.tile([128, TOK], F32R, tag="gsq")
                nc.scalar.activation(gsq[:], g[:], AF.Square)
                g_tiles[ke] = g
                gsq_tiles[ke] = gsq
                if ke >= 1:
                    mm2_pair(ke - 1)
            mm2_pair(KE - 1)

            # ---- evacuate q/S1/S2, transpose to token-major ----
            # S2 copies to partition 32 (engine ops need start partition 0/32/64/96)
            qsb = ep.tile([33, TOK], F32, tag="qsb")
            nc.vector.tensor_copy(qsb[0:OUT2, :], pq[:])
            nc.vector.tensor_copy(qsb[32:33, :], ps2[:])
            pt = ep.tile([128, NB, 25], F32, tag="pt")
            for b in range(NB):
                ptr = ptrp.tile([128, 25], F32, tag="ptr")
                sl = slice(b * 128, (b + 1) * 128)
                nc.tensor.transpose(ptr[:, 0:24], qsb[0:24, sl], id_sb[0:24, 0:24])
                nc.tensor.transpose(ptr[:, 24:25], qsb[32:33, sl],
                                    id_sb[32:33, 32:33])
                nc.vector.tensor_copy(pt[:, b, :], ptr[:, 0:25])

            # ---- epilogue (batched over all NB blocks via strided APs) ----
            TT = nc.vector.tensor_tensor
            TS = nc.vector.tensor_scalar
            RED = nc.vector.tensor_reduce
            # rsqrt on DVE (Quake seed + 2 Newton): keeps ACT on one table set
            magic = ep.tile([128, 8], U32, tag="magic")
            nc.vector.memset(magic[:], QMAGIC)

            def RSQRT(dst, src, n, eps):
                if eps:
                    TS(src, src, float(eps), None, ALU.add)
                t = ep.tile([128, n], F32, tag="rsq_t")
                s = ep.tile([128, n], F32, tag="rsq_s")
                nc.vector.tensor_scalar(t[:].bitcast(U32), src.bitcast(U32),
                                        1, None, ALU.logical_shift_right)
                nc.vector.tensor_tensor(s[:].bitcast(U32), magic[:, 0:n],
                                        t[:].bitcast(U32), ALU.subtract)
                for _ in range(2):
                    TT(t[:], s[:], s[:], ALU.mult)
                    TT(t[:], t[:], src, ALU.mult)
                    TS(t[:], t[:], -0.5, 1.5, ALU.mult, ALU.add)
                    TT(s[:], s[:], t[:], ALU.mult)
                nc.vector.tensor_copy(dst, s[:])

            # LayerNorm stats: mu = S1/D (col 23), msq = S2/D (col 24)
            mus = ep.tile([128, NB, 2], F32, tag="mus")
            TS(mus[:], pt[:, :, 23:25], 1.0 / D, None, ALU.mult)
            mu = mus[:, :, 0]
            var = ep.tile([128, NB], F32, tag="var")
            TT(var[:], mu, mu, ALU.mult)
            TT(var[:], mus[:, :, 1], var[:], ALU.subtract)
            invsig = ep.tile([128, NB], F32, tag="invsig")
            RSQRT(invsig[:], var[:], NB, LN_EPS)
            sig = ep.tile([128, NB], F32, tag="sig")
            TT(sig[:], var[:], invsig[:], ALU.mult)               # sigma = var*rsqrt(var)

            # pp = q - mu*c1 (un-normalized p); trans = pp[0:3]*invsig*10
            pp = ep.tile([128, NB, 23], F32, tag="pp")
            TT(pp[:], _bc(mu, 2, 23), _bc(c1b_sb[:], 1, NB), ALU.mult)
            TT(pp[:], pt[:, :, 0:23], pp[:], ALU.subtract)
            if c3_nonzero:
                # need true p for correctness when c3 != 0: p = pp*invsig + c3
                TT(pp[:], pp[:], _bc(invsig[:], 2, 23), ALU.mult)
                TT(pp[:], pp[:], _bc(c3b_sb[:], 1, NB), ALU.add)
            ti = ep.tile([128, NB], F32, tag="ti")
            if c3_nonzero:
                nc.vector.memset(ti[:], TRANS_SCALE)
            else:
                TS(ti[:], invsig[:], TRANS_SCALE, None, ALU.mult)
            tr = ep.tile([128, NB, 3], F32, tag="tr")
            TT(tr[:], pp[:, :, 0:3], _bc(ti[:], 2, 3), ALU.mult)

            # xv = ppx/(|ppx| + eps*sigma'), sigma' = sigma (or 1 if c3 path)
            t3a = ep.tile([128, NB, 3], F32, tag="t3a")
            t3b = ep.tile([128, NB, 3], F32, tag="t3b")
            nrm2 = ep.tile([128, 2, NB], F32, tag="nrm2")
            TT(t3a[:], pp[:, :, 3:6], pp[:, :, 3:6], ALU.mult)
            RED(nrm2[:, 0, :], t3a[:], AX.X, ALU.add)
            TT(t3a[:], pp[:, :, 6:9], pp[:, :, 6:9], ALU.mult)
            RED(nrm2[:, 1, :], t3a[:], AX.X, ALU.add)
            r8 = ep.tile([128, 2, NB], F32, tag="r8")
            RSQRT(r8[:].rearrange("p a b -> p (a b)"),
                  nrm2[:].rearrange("p a b -> p (a b)"), 2 * NB, 0.0)
            s8 = ep.tile([128, 2, NB], F32, tag="s8")
            TT(s8[:], nrm2[:], r8[:], ALU.mult)
            se4 = ep.tile([128, NB], F32, tag="se4")
            if c3_nonzero:
                nc.vector.memset(se4[:], NORM_EPS)
            else:
                TS(se4[:], sig[:], NORM_EPS, None, ALU.mult)
            TT(s8[:], s8[:], _bc(se4[:], 1, 2), ALU.add)
            inv8 = ep.tile([128, 2, NB], F32, tag="inv8")
            nc.vector.reciprocal(inv8[:], s8[:])
            xv = ep.tile([128, NB, 3], F32, tag="xv")
            yv = ep.tile([128, NB, 3], F32, tag="yv")
            TT(xv[:], pp[:, :, 3:6], _bc(inv8[:, 0, :], 2, 3), ALU.mult)
            TT(yv[:], pp[:, :, 6:9], _bc(inv8[:, 1, :], 2, 3), ALU.mult)

            # Gram-Schmidt on (-xv, yv); Ru columns [e0 e1 e2]
            Ru = ep.tile([128, NB, 3, 3], F32, tag="Ru")
            u4 = ep.tile([128, NB], F32, tag="u4")
            TT(t3a[:], xv[:], xv[:], ALU.mult)
            RED(u4[:], t3a[:], AX.X, ALU.add)
            r0 = ep.tile([128, NB], F32, tag="r0")
            RSQRT(r0[:], u4[:], NB, GS_EPS)
            TS(r0[:], r0[:], -1.0, None, ALU.mult)
            TT(Ru[:, :, :, 0], xv[:], _bc(r0[:], 2, 3), ALU.mult)        # e0
            d4 = ep.tile([128, NB], F32, tag="d4")
            TT(t3a[:], Ru[:, :, :, 0], yv[:], ALU.mult)
            RED(d4[:], t3a[:], AX.X, ALU.add)
            TT(t3a[:], Ru[:, :, :, 0], _bc(d4[:], 2, 3), ALU.mult)
            e1u = ep.tile([128, NB, 3], F32, tag="e1u")
            TT(e1u[:], yv[:], t3a[:], ALU.subtract)
            TT(t3a[:], e1u[:], e1u[:], ALU.mult)
            n14 = ep.tile([128, NB], F32, tag="n14")
            RED(n14[:], t3a[:], AX.X, ALU.add)
            r1 = ep.tile([128, NB], F32, tag="r1")
            RSQRT(r1[:], n14[:], NB, GS_EPS)
            TT(Ru[:, :, :, 1], e1u[:], _bc(r1[:], 2, 3), ALU.mult)       # e1
            ca = ep.tile([128, NB], F32, tag="ca")
            cb = ep.tile([128, NB], F32, tag="cb")
            for j, (a, b_) in enumerate([(1, 2), (2, 0), (0, 1)]):       # e2 = e0 x e1
                TT(ca[:], Ru[:, :, a, 0], Ru[:, :, b_, 1], ALU.mult)
                TT(cb[:], Ru[:, :, b_, 0], Ru[:, :, a, 1], ALU.mult)
                TT(Ru[:, :, j, 2], ca[:], cb[:], ALU.subtract)

            # mask: Rm = (Ru - I)*m + I ; tu = tr*m
            Ruf = Ru[:].rearrange("p b i j -> p b (i j)")
            Rm = ep.tile([128, NB, 9], F32, tag="Rm")
            TT(Rm[:], Ruf, _bc(i9b_sb[:], 1, NB), ALU.subtract)
            TT(Rm[:], Rm[:], _bc(msk_sb[:], 2, 9), ALU.mult)
            TT(Rm[:], Rm[:], _bc(i9b_sb[:], 1, NB), ALU.add)
            tu = ep.tile([128, NB, 3], F32, tag="tu")
            TT(tu[:], tr[:], _bc(msk_sb[:], 2, 3), ALU.mult)

            # compose: R = R0 @ Rm ; tvec = R0 @ tu + t0
            affo_sb = ep.tile([128, NB, 12], F32, tag="affo")
            xyz_sb = ep.tile([128, NB, 9], F32, tag="xyz")
            R0v = aff_sb[:, :, 0:9].rearrange("p b (i j) -> p b i j", i=3)
            Rmv = Rm[:].rearrange("p b (j k) -> p b j k", j=3)
            Rov = affo_sb[:, :, 0:9].rearrange("p b (i k) -> p b i k", i=3)
            t33 = ep.tile([128, NB, 3, 3], F32, tag="t33")
            for j in range(3):
                dst = Rov if j == 0 else t33[:]
                TT(dst, _bc(R0v[:, :, :, j], 3, 3), _bc(Rmv[:, :, j, :], 2, 3),
                   ALU.mult)
                if j > 0:
                    TT(Rov, Rov, t33[:], ALU.add)
            tv = affo_sb[:, :, 9:12]
            for j in range(3):
                dst = tv if j == 0 else t3a[:]
                TT(dst, R0v[:, :, :, j], _bc(tu[:, :, j], 2, 3), ALU.mult)
                if j > 0:
                    TT(tv, tv, t3a[:], ALU.add)
            TT(tv, tv, aff_sb[:, :, 9:12], ALU.add)

            # backbone atoms: N = -0.525*Rc0 + 1.363*Rc1 + t ; CA = t ; C = 1.526*Rc0 + t
            xyzv = xyz_sb[:].rearrange("p b (a i) -> p b a i", a=3)
            TS(t3a[:], Rov[:, :, :, 0], -0.525, None, ALU.mult)
            TS(t3b[:], Rov[:, :, :, 1], 1.363, None, ALU.mult)
            TT(t3a[:], t3a[:], t3b[:], ALU.add)
            TT(xyzv[:, :, 0, :], t3a[:], tv, ALU.add)
            nc.vector.tensor_copy(xyzv[:, :, 1, :], tv)
            TS(t3b[:], Rov[:, :, :, 0], 1.526, None, ALU.mult)
            TT(xyzv[:, :, 2, :], t3b[:], tv, ALU.add)

            nc.sync.dma_start(affo_d[:], affo_sb[:])
            nc.sync.dma_start(xyzo_d[:], xyz_sb[:])

    nc.compile()
    return nc


_CACHE = {}


def _get_program(c3_nonzero):
    key = c3_nonzero
    if key not in _CACHE:
        _CACHE[key] = build_program(c3_nonzero)
    return _CACHE[key]


def host_prep(x, affine, affine_mask, w1, b1, ln_g, ln_b, w2, b2):
    x = np.asarray(x, np.float32)
    affine = np.asarray(affine, np.float32)
    w1 = np.asarray(w1, np.float32)
    b1 = np.asarray(b1, np.float32)
    ln_g = np.asarray(ln_g, np.float32)
    ln_b = np.asarray(ln_b, np.float32)
    w2 = np.asarray(w2, np.float32)
    b2 = np.asarray(b2, np.float32)

    # host-side prep of replicated params
    # w1r[ke, kd, p, f] = w1[128*ke+f, 128*kd+p]
    w1r = np.ascontiguousarray(w1.reshape(KE, 128, KD, 128).transpose(0, 2, 3, 1))
    a2 = w2 * ln_g[None, :]                                      # [23, 1536]
    w2c = np.concatenate([a2, np.ones((1, D), np.float32)], 0)   # [24, 1536]
    # w2c_sb[p, ke, o] = w2c[o, 128*ke+p]
    w2c = np.ascontiguousarray(w2c.T.reshape(KE, 128, OUT2).transpose(1, 0, 2))
    ones1 = np.ones((128, 1), np.float32)
    b1c = np.ascontiguousarray(b1.reshape(KE, 128).T)            # [128, ke]
    c1 = a2.sum(1).astype(np.float32)                            # [23]
    c3 = (w2 @ ln_b + b2).astype(np.float32)                     # [23]
    c1b = np.broadcast_to(c1, (128, 23)).copy()
    c3b = np.broadcast_to(c3, (128, 23)).copy()
    i9b = np.broadcast_to(np.eye(3, dtype=np.float32).reshape(9), (128, 9)).copy()
    ident = np.eye(128, dtype=np.float32)
    c3_nonzero = bool(np.abs(c3).max() > 0)

    xf = x.reshape(B * L, D)
    af = affine.reshape(B * L, 12)
    mf = np.asarray(affine_mask).reshape(B * L).astype(np.float32)

    in_maps = []
    for s in range(NCORES):
        sl = slice(s * TOK, (s + 1) * TOK)
        # xr[p, kd, t] = x[t, 128*kd+p]
        xs = np.ascontiguousarray(
            xf[sl].T.reshape(KD, 128, TOK).transpose(1, 0, 2))
        affs = np.ascontiguousarray(af[sl].reshape(NB, 128, 12).transpose(1, 0, 2))
        msks = np.ascontiguousarray(mf[sl].reshape(NB, 128).T)
        in_maps.append({
            "xr": xs, "w1r": w1r, "w2c": w2c, "ones1": ones1, "b1c": b1c,
            "c1b": c1b, "c3b": c3b, "i9b": i9b, "ident": ident,
            "aff": affs, "maskf": msks,
        })
    return in_maps, c3_nonzero


def kernel(x, affine, affine_mask, w1, b1, ln_g, ln_b, w2, b2, _trace=False):
    in_maps, c3_nonzero = host_prep(x, affine, affine_mask, w1, b1, ln_g,
                                    ln_b, w2, b2)
    nc = _get_program(c3_nonzero)
    # The axon/8-core execution path fails intermittently (~50%/exec, loud
    # JaxRuntimeError, never silent corruption); failed runs recover on
    # retry within the same process, so retry until it sticks.
    last_err = None
    for attempt in range(10):
        try:
            res = run_bass_kernel_spmd(nc, in_maps,
                                       core_ids=list(range(NCORES)),
                                       trace=_trace and attempt < 2)
            break
        except Exception as e:   # noqa: BLE001
            last_err = e
    else:
        raise last_err

    aff_out = np.empty((B * L, 12), np.float32)
    xyz_out = np.empty((B * L, 9), np.float32)
    for s in range(NCORES):
        sl = slice(s * TOK, (s + 1) * TOK)
        r = res.results[s]
        aff_out[sl] = r["aff_out"].transpose(1, 0, 2).reshape(TOK, 12)
        xyz_out[sl] = r["xyz_out"].transpose(1, 0, 2).reshape(TOK, 9)

    out_affine = aff_out.reshape(B, L, 12)
    out_xyz = xyz_out.reshape(B, L, 3, 3)
    kernel._last_result = res
    return out_affine, out_xyz


# revision 39
# speedup vs baseline: 1.1674x; 1.1674x over previous
"""Dim6RotStructureHead Trainium2 kernel.

Sharding: pure data parallel — 4096 tokens split 512/core across 8 cores,
all params replicated.

Per-core pipeline (hT layout: features on partitions, tokens on free axis):
  mm1:  h_pre[e,t] = sum_d w1T[d,e] . xT[d,t]      (fp32r PE, 12x12 blocks)
  gelu: g = Gelu(h_pre + b1)                        (ACT, fused bias)
  gsq = g*g                                         (DVE)
  mm2:  q[o,t]  = sum_e (w2*ln_g)T[e,o] . g[e,t]   (+ ones col -> S1)
  s2:   S2[t]   = sum_e gsq[e,t]                    (ones lhsT)
  LayerNorm deferred past mm2: p = (q - mu*c1) * inv_sigma (+ c3); the
  direction vectors are normalized scale-invariantly:
  xv = pp/(|pp| + eps*sigma) with pp = q - mu*c1.
  Small PE transposes bring [24,512]+[1,512] -> [128 tok, 25]; geometry
  epilogue (Gram-Schmidt simplifies: trans cancels -> GS(-xv, yv), mask,
  compose, backbone coords) runs batched on DVE + ACT sqrt.
"""
import numpy as np

import concourse.bacc as bacc
import concourse.mybir as mybir
import concourse.tile as tile
from concourse.bass_utils import run_bass_kernel_spmd

F32 = mybir.dt.float32
F32R = mybir.dt.float32r
U32 = mybir.dt.uint32
AF = mybir.ActivationFunctionType
ALU = mybir.AluOpType
AX = mybir.AxisListType
QMAGIC = 0x5F3759DF

B, L, D = 4, 1024, 1536
NCORES = 8
TOK = (B * L) // NCORES          # 512 tokens per core
KD = D // 128                    # 12 contraction blocks
KE = D // 128                    # 12 feature blocks
NB = TOK // 128                  # 4 token blocks per core
OUT2 = 24                        # 23 proj outputs + ones column (S1)
TRANS_SCALE = 10.0
LN_EPS = 1e-5
NORM_EPS = 1e-5
GS_EPS = 1e-12


def _bc(ap, axis, n):
    """Broadcast AP along a new axis (step-0)."""
    ap = ap.unsqueeze(axis)
    shape = list(ap.shape)
    shape[axis] = n
    return ap.broadcast_to(shape)


def build_program(c3_nonzero: bool):
    nc = bacc.Bacc("TRN2", target_bir_lowering=False, debug=False,
                   enable_asserts=False, num_devices=NCORES)

    # ---- DRAM I/O (per core) ----
    # x: [p][kd][t] so every partition reads one contiguous run per DMA
    xr_d = nc.dram_tensor("xr", [128, KD, TOK], F32R, kind="ExternalInput").ap()
    # w1: [ke][p][kd*128] -> 6KB contiguous per partition per ke-block
    w1r_d = nc.dram_tensor("w1r", [KE, 128, KD * 128], F32R, kind="ExternalInput").ap()
    w2c_d = nc.dram_tensor("w2c", [128, KE, OUT2], F32R, kind="ExternalInput").ap()
    ones_d = nc.dram_tensor("ones1", [128, 1], F32R, kind="ExternalInput").ap()
    b1c_d = nc.dram_tensor("b1c", [128, KE], F32, kind="ExternalInput").ap()
    c1b_d = nc.dram_tensor("c1b", [128, 23], F32, kind="ExternalInput").ap()
    c3b_d = nc.dram_tensor("c3b", [128, 23], F32, kind="ExternalInput").ap()
    i9b_d = nc.dram_tensor("i9b", [128, 9], F32, kind="ExternalInput").ap()
    ident_d = nc.dram_tensor("ident", [128, 128], F32, kind="ExternalInput").ap()
    aff_d = nc.dram_tensor("aff", [128, NB, 12], F32, kind="ExternalInput").ap()
    msk_d = nc.dram_tensor("maskf", [128, NB], F32, kind="ExternalInput").ap()
    affo_d = nc.dram_tensor("aff_out", [128, NB, 12], F32, kind="ExternalOutput").ap()
    xyzo_d = nc.dram_tensor("xyz_out", [128, NB, 9], F32, kind="ExternalOutput").ap()

    with tile.TileContext(nc) as tc:
        with tc.tile_pool(name="wgt", bufs=1) as wgt, \
             tc.tile_pool(name="xin", bufs=1) as xin, \
             tc.tile_pool(name="cst", bufs=1) as cst, \
             tc.tile_pool(name="gp", bufs=3) as gp, \
             tc.tile_pool(name="sq", bufs=3) as sqp, \
             tc.tile_pool(name="ep", bufs=1) as ep, \
             tc.tile_pool(name="ph", bufs=3, space="PSUM") as php, \
             tc.tile_pool(name="pq", bufs=1, space="PSUM") as pqp, \
             tc.tile_pool(name="ps2", bufs=1, space="PSUM") as ps2p, \
             tc.tile_pool(name="ptr", bufs=2, space="PSUM") as ptrp:

            # ---- small/replicated inputs ----
            w2c_sb = cst.tile([128, KE, OUT2], F32R, tag="w2c")
            nc.sync.dma_start(w2c_sb[:], w2c_d[:])
            ones_sb = cst.tile([128, 1], F32R, tag="ones")
            nc.sync.dma_start(ones_sb[:], ones_d[:])
            b1_sb = cst.tile([128, KE], F32, tag="b1")
            nc.sync.dma_start(b1_sb[:], b1c_d[:])
            c1b_sb = cst.tile([128, 23], F32, tag="c1b")
            nc.sync.dma_start(c1b_sb[:], c1b_d[:])
            c3b_sb = cst.tile([128, 23], F32, tag="c3b")
            if c3_nonzero:
                nc.sync.dma_start(c3b_sb[:], c3b_d[:])
            i9b_sb = cst.tile([128, 9], F32, tag="i9b")
            nc.sync.dma_start(i9b_sb[:], i9b_d[:])
            id_sb = cst.tile([128, 128], F32, tag="ident")
            nc.sync.dma_start(id_sb[:], ident_d[:])
            aff_sb = cst.tile([128, NB, 12], F32, tag="aff")
            nc.sync.dma_start(aff_sb[:], aff_d[:])
            msk_sb = cst.tile([128, NB], F32, tag="msk")
            nc.sync.dma_start(msk_sb[:], msk_d[:])

            # ---- big inputs: x halves interleaved with first w1 blocks ----
            x_sb = xin.tile([128, KD, TOK], F32R, tag="x")
            w_sb = wgt.tile([128, KE, KD, 128], F32R, tag="w1")
            HK = KD // 2
            nc.sync.dma_start(x_sb[:, 0:HK, :], xr_d[:, 0:HK, :])
            nc.sync.dma_start(w_sb[:, 0],
                              w1r_d[0].rearrange("p (k f) -> p k f", k=KD))
            nc.sync.dma_start(x_sb[:, HK:KD, :], xr_d[:, HK:KD, :])
            for ke in range(1, KE):
                nc.sync.dma_start(w_sb[:, ke],
                                  w1r_d[ke].rearrange("p (k f) -> p k f", k=KD))

            # ---- main loop ----
            pq = pqp.tile([OUT2, TOK], F32, tag="pq")
            ps2 = ps2p.tile([1, TOK], F32, tag="ps2")
            g_tiles = {}
            gsq_tiles = {}

            def mm2_pair(ke):
                nc.tensor.matmul(pq[:], w2c_sb[:, ke, :], g_tiles.pop(ke)[:],
                                 start=(ke == 0), stop=(ke == KE - 1))
                nc.tensor.matmul(ps2[:], ones_sb[:], gsq_tiles.pop(ke)[:],
                                 start=(ke == 0), stop=(ke == KE - 1))

            for ke in range(KE):
                ph = php.tile([128, TOK], F32, tag="ph")
                for kd in range(KD):
                    nc.tensor.matmul(ph[:], w_sb[:, ke, kd, :], x_sb[:, kd, :],
                                     start=(kd == 0), stop=(kd == KD - 1))
                g = gp.tile([128, TOK], F32R, tag="g")
                nc.scalar.activation(g[:], ph[:], AF.Gelu, bias=b1_sb[:, ke:ke + 1])
                gsq = sqp.tile([128, TOK], F32R, tag="gsq")
                nc.scalar.activation(gsq[:], g[:], AF.Square)
                g_tiles[ke] = g
                gsq_tiles[ke] = gsq
                if ke >= 1:
                    mm2_pair(ke - 1)
            mm2_pair(KE - 1)

            # ---- evacuate q/S1/S2, transpose to token-major ----
            # S2 copies to partition 32 (engine ops need start partition 0/32/64/96)
            qsb = ep.tile([33, TOK], F32, tag="qsb")
            nc.vector.tensor_copy(qsb[0:OUT2, :], pq[:])
            nc.vector.tensor_copy(qsb[32:33, :], ps2[:])
            pt = ep.tile([128, NB, 25], F32, tag="pt")
            for b in range(NB):
                ptr = ptrp.tile([128, 25], F32, tag="ptr")
                sl = slice(b * 128, (b + 1) * 128)
                nc.tensor.transpose(ptr[:, 0:24], qsb[0:24, sl], id_sb[0:24, 0:24])
                nc.tensor.transpose(ptr[:, 24:25], qsb[32:33, sl],
                                    id_sb[32:33, 32:33])
                nc.vector.tensor_copy(pt[:, b, :], ptr[:, 0:25])

            # ---- epilogue (batched over all NB blocks via strided APs) ----
            TT = nc.vector.tensor_tensor
            TS = nc.vector.tensor_scalar
            RED = nc.vector.tensor_reduce
            def SQRT(dst, src, eps):
                # eps-add on DVE so ACT only ever runs plain Sqrt (bias 0.0)
                if eps:
                    TS(src, src, float(eps), None, ALU.add)
                nc.scalar.activation(dst, src, AF.Sqrt)

            # LayerNorm stats: mu = S1/D (col 23), msq = S2/D (col 24)
            mus = ep.tile([128, NB, 2], F32, tag="mus")
            TS(mus[:], pt[:, :, 23:25], 1.0 / D, None, ALU.mult)
            mu = mus[:, :, 0]
            var = ep.tile([128, NB], F32, tag="var")
            TT(var[:], mu, mu, ALU.mult)
            TT(var[:], mus[:, :, 1], var[:], ALU.subtract)
            sig = ep.tile([128, NB], F32, tag="sig")
            SQRT(sig[:], var[:], LN_EPS)                          # sigma
            invsig = ep.tile([128, NB], F32, tag="invsig")
            nc.vector.reciprocal(invsig[:], sig[:])

            # pp = q - mu*c1 (un-normalized p); trans = pp[0:3]*invsig*10
            pp = ep.tile([128, NB, 23], F32, tag="pp")
            TT(pp[:], _bc(mu, 2, 23), _bc(c1b_sb[:], 1, NB), ALU.mult)
            TT(pp[:], pt[:, :, 0:23], pp[:], ALU.subtract)
            if c3_nonzero:
                # need true p for correctness when c3 != 0: p = pp*invsig + c3
                TT(pp[:], pp[:], _bc(invsig[:], 2, 23), ALU.mult)
                TT(pp[:], pp[:], _bc(c3b_sb[:], 1, NB), ALU.add)
            ti = ep.tile([128, NB], F32, tag="ti")
            if c3_nonzero:
                nc.vector.memset(ti[:], TRANS_SCALE)
            else:
                TS(ti[:], invsig[:], TRANS_SCALE, None, ALU.mult)
            tr = ep.tile([128, NB, 3], F32, tag="tr")
            TT(tr[:], pp[:, :, 0:3], _bc(ti[:], 2, 3), ALU.mult)

            # xv = ppx/(|ppx| + eps*sigma'), sigma' = sigma (or 1 if c3 path)
            t3a = ep.tile([128, NB, 3], F32, tag="t3a")
            t3b = ep.tile([128, NB, 3], F32, tag="t3b")
            nrm2 = ep.tile([128, 2, NB], F32, tag="nrm2")
            TT(t3a[:], pp[:, :, 3:6], pp[:, :, 3:6], ALU.mult)
            RED(nrm2[:, 0, :], t3a[:], AX.X, ALU.add)
            TT(t3a[:], pp[:, :, 6:9], pp[:, :, 6:9], ALU.mult)
            RED(nrm2[:, 1, :], t3a[:], AX.X, ALU.add)
            s8 = ep.tile([128, 2, NB], F32, tag="s8")
            SQRT(s8[:], nrm2[:], 0.0)
            se4 = ep.tile([128, NB], F32, tag="se4")
            if c3_nonzero:
                nc.vector.memset(se4[:], NORM_EPS)
            else:
                TS(se4[:], sig[:], NORM_EPS, None, ALU.mult)
            TT(s8[:], s8[:], _bc(se4[:], 1, 2), ALU.add)
            inv8 = ep.tile([128, 2, NB], F32, tag="inv8")
            nc.vector.reciprocal(inv8[:], s8[:])
            xv = ep.tile([128, NB, 3], F32, tag="xv")
            yv = ep.tile([128, NB, 3], F32, tag="yv")
            TT(xv[:], pp[:, :, 3:6], _bc(inv8[:, 0, :], 2, 3), ALU.mult)
            TT(yv[:], pp[:, :, 6:9], _bc(inv8[:, 1, :], 2, 3), ALU.mult)

            # Gram-Schmidt on (-xv, yv); Ru columns [e0 e1 e2]
            Ru = ep.tile([128, NB, 3, 3], F32, tag="Ru")
            u4 = ep.tile([128, NB], F32, tag="u4")
            TT(t3a[:], xv[:], xv[:], ALU.mult)
            RED(u4[:], t3a[:], AX.X, ALU.add)
            su = ep.tile([128, NB], F32, tag="su")
            SQRT(su[:], u4[:], GS_EPS)
            r0 = ep.tile([128, NB], F32, tag="r0")
            nc.vector.reciprocal(r0[:], su[:])
            TS(r0[:], r0[:], -1.0, None, ALU.mult)
            TT(Ru[:, :, :, 0], xv[:], _bc(r0[:], 2, 3), ALU.mult)        # e0
            d4 = ep.tile([128, NB], F32, tag="d4")
            TT(t3a[:], Ru[:, :, :, 0], yv[:], ALU.mult)
            RED(d4[:], t3a[:], AX.X, ALU.add)
            TT(t3a[:], Ru[:, :, :, 0], _bc(d4[:], 2, 3), ALU.mult)
            e1u = ep.tile([128, NB, 3], F32, tag="e1u")
            TT(e1u[:], yv[:], t3a[:], ALU.subtract)
            TT(t3a[:], e1u[:], e1u[:], ALU.mult)
            n14 = ep.tile([128, NB], F32, tag="n14")
            RED(n14[:], t3a[:], AX.X, ALU.add)
            sn = ep.tile([128, NB], F32, tag="sn")
            SQRT(sn[:], n14[:], GS_EPS)
            r1 = ep.tile([128, NB], F32, tag="r1")
            nc.vector.reciprocal(r1[:], sn[:])
            TT(Ru[:, :, :, 1], e1u[:], _bc(r1[:], 2, 3), ALU.mult)       # e1
            ca = ep.tile([128, NB], F32, tag="ca")
            cb = ep.tile([128, NB], F32, tag="cb")
            for j, (a, b_) in enumerate([(1, 2), (2, 0), (0, 1)]):       # e2 = e0 x e1
                TT(ca[:], Ru[:, :, a, 0], Ru[:, :, b_, 1], ALU.mult)
                TT(cb[:], Ru[:, :, b_, 0], Ru[:, :, a, 1], ALU.mult)
                TT(Ru[:, :, j, 2], ca[:], cb[:], ALU.subtract)

            # mask: Rm = (Ru - I)*m + I ; tu = tr*m
            Ruf = Ru[:].rearrange("p b i j -> p b (i j)")
            Rm = ep.tile([128, NB, 9], F32, tag="Rm")
            TT(Rm[:], Ruf, _bc(i9b_sb[:], 1, NB), ALU.subtract)
            TT(Rm[:], Rm[:], _bc(msk_sb[:], 2, 9), ALU.mult)
            TT(Rm[:], Rm[:], _bc(i9b_sb[:], 1, NB), ALU.add)
            tu = ep.tile([128, NB, 3], F32, tag="tu")
            TT(tu[:], tr[:], _bc(msk_sb[:], 2, 3), ALU.mult)

            # compose: R = R0 @ Rm ; tvec = R0 @ tu + t0
            affo_sb = ep.tile([128, NB, 12], F32, tag="affo")
            xyz_sb = ep.tile([128, NB, 9], F32, tag="xyz")
            R0v = aff_sb[:, :, 0:9].rearrange("p b (i j) -> p b i j", i=3)
            Rmv = Rm[:].rearrange("p b (j k) -> p b j k", j=3)
            Rov = affo_sb[:, :, 0:9].rearrange("p b (i k) -> p b i k", i=3)
            t33 = ep.tile([128, NB, 3, 3], F32, tag="t33")
            for j in range(3):
                dst = Rov if j == 0 else t33[:]
                TT(dst, _bc(R0v[:, :, :, j], 3, 3), _bc(Rmv[:, :, j, :], 2, 3),
                   ALU.mult)
                if j > 0:
                    TT(Rov, Rov, t33[:], ALU.add)
            tv = affo_sb[:, :, 9:12]
            for j in range(3):
                dst = tv if j == 0 else t3a[:]
                TT(dst, R0v[:, :, :, j], _bc(tu[:, :, j], 2, 3), ALU.mult)
                if j > 0:
                    TT(tv, tv, t3a[:], ALU.add)
            TT(tv, tv, aff_sb[:, :, 9:12], ALU.add)

            # backbone atoms: N = -0.525*Rc0 + 1.363*Rc1 + t ; CA = t ; C = 1.526*Rc0 + t
            xyzv = xyz_sb[:].rearrange("p b (a i) -> p b a i", a=3)
            TS(t3a[:], Rov[:, :, :, 0], -0.525, None, ALU.mult)
            TS(t3b[:], Rov[:, :, :, 1], 1.363, None, ALU.mult)
            TT(t3a[:], t3a[:], t3b[:], ALU.add)
            TT(xyzv[:, :, 0, :], t3a[:], tv, ALU.add)
            nc.vector.tensor_copy(xyzv[:, :, 1, :], tv)
            TS(t3b[:], Rov[:, :, :, 0], 1.526, None, ALU.mult)
            TT(xyzv[:, :, 2, :], t3b[:], tv, ALU.add)

            nc.sync.dma_start(affo_d[:], affo_sb[:])
            nc.sync.dma_start(xyzo_d[:], xyz_sb[:])

    nc.compile()
    return nc


_CACHE = {}


def _get_program(c3_nonzero):
    key = c3_nonzero
    if key not in _CACHE:
        _CACHE[key] = build_program(c3_nonzero)
    return _CACHE[key]


def host_prep(x, affine, affine_mask, w1, b1, ln_g, ln_b, w2, b2):
    x = np.asarray(x, np.float32)
    affine = np.asarray(affine, np.float32)
    w1 = np.asarray(w1, np.float32)
    b1 = np.asarray(b1, np.float32)
    ln_g = np.asarray(ln_g, np.float32)
    ln_b = np.asarray(ln_b, np.float32)
    w2 = np.asarray(w2, np.float32)
    b2 = np.asarray(b2, np.float32)

    # host-side prep of replicated params
    # w1r[ke, p, kd*128+f] = w1[128*ke+f, 128*kd+p]
    w1r = np.ascontiguousarray(
        w1.reshape(KE, 128, KD, 128).transpose(0, 3, 2, 1).reshape(KE, 128, KD * 128))
    a2 = w2 * ln_g[None, :]                                      # [23, 1536]
    w2c = np.concatenate([a2, np.ones((1, D), np.float32)], 0)   # [24, 1536]
    # w2c_sb[p, ke, o] = w2c[o, 128*ke+p]
    w2c = np.ascontiguousarray(w2c.T.reshape(KE, 128, OUT2).transpose(1, 0, 2))
    ones1 = np.ones((128, 1), np.float32)
    b1c = np.ascontiguousarray(b1.reshape(KE, 128).T)            # [128, ke]
    c1 = a2.sum(1).astype(np.float32)                            # [23]
    c3 = (w2 @ ln_b + b2).astype(np.float32)                     # [23]
    c1b = np.broadcast_to(c1, (128, 23)).copy()
    c3b = np.broadcast_to(c3, (128, 23)).copy()
    i9b = np.broadcast_to(np.eye(3, dtype=np.float32).reshape(9), (128, 9)).copy()
    ident = np.eye(128, dtype=np.float32)
    c3_nonzero = bool(np.abs(c3).max() > 0)

    xf = x.reshape(B * L, D)
    af = affine.reshape(B * L, 12)
    mf = np.asarray(affine_mask).reshape(B * L).astype(np.float32)

    in_maps = []
    for s in range(NCORES):
        sl = slice(s * TOK, (s + 1) * TOK)
        # xr[p, kd, t] = x[t, 128*kd+p]
        xs = np.ascontiguousarray(
            xf[sl].T.reshape(KD, 128, TOK).transpose(1, 0, 2))
        affs = np.ascontiguousarray(af[sl].reshape(NB, 128, 12).transpose(1, 0, 2))
        msks = np.ascontiguousarray(mf[sl].reshape(NB, 128).T)
        in_maps.append({
            "xr": xs, "w1r": w1r, "w2c": w2c, "ones1": ones1, "b1c": b1c,
            "c1b": c1b, "c3b": c3b, "i9b": i9b, "ident": ident,
            "aff": affs, "maskf": msks,
        })
    return in_maps, c3_nonzero


def kernel(x, affine, affine_mask, w1, b1, ln_g, ln_b, w2, b2, _trace=False):
    in_maps, c3_nonzero = host_prep(x, affine, affine_mask, w1, b1, ln_g,
                                    ln_b, w2, b2)
    nc = _get_program(c3_nonzero)
    # The axon/8-core execution path fails intermittently (~50%/exec, loud
    # JaxRuntimeError, never silent corruption); failed runs recover on
    # retry within the same process, so retry until it sticks.
    last_err = None
    for attempt in range(10):
        try:
            res = run_bass_kernel_spmd(nc, in_maps,
                                       core_ids=list(range(NCORES)),
                                       trace=_trace and attempt == 0)
            break
        except Exception as e:   # noqa: BLE001
            last_err = e
            try:
                # A failed traced attempt can leave an NRT profile session
                # open terminal-side; stop it so retries aren't poisoned.
                import ctypes
                import tempfile
                lib = ctypes.CDLL("/opt/axon/libaxon_pjrt.so")
                lib.axon_stop_nrt_profile.argtypes = [ctypes.c_char_p]
                lib.axon_stop_nrt_profile.restype = ctypes.c_int64
                lib.axon_stop_nrt_profile(tempfile.mkdtemp().encode())
            except Exception:    # noqa: BLE001
                pass
            try:
                import jax
                jax.clear_caches()
            except Exception:    # noqa: BLE001
                pass
    else:
        raise last_err

    aff_out = np.empty((B * L, 12), np.float32)
    xyz_out = np.empty((B * L, 9), np.float32)
    for s in range(NCORES):
        sl = slice(s * TOK, (s + 1) * TOK)
        r = res.results[s]
        aff_out[sl] = r["aff_out"].transpose(1, 0, 2).reshape(TOK, 12)
        xyz_out[sl] = r["xyz_out"].transpose(1, 0, 2).reshape(TOK, 9)

    out_affine = aff_out.reshape(B, L, 12)
    out_xyz = xyz_out.reshape(B, L, 3, 3)
    kernel._last_result = res
    return out_affine, out_xyz
